# revision 31
# baseline (speedup 1.0000x reference)
"""CollaborativeAttention Trainium2 kernel (fp8 DoubleRow everywhere).

Sharding: 8 cores = (batch b in {0,1}) x (512-query-row block). Each core
computes its 512 output rows end to end; k/v/content-bias are computed
redundantly within each batch group, so no device collectives are needed.
Each core's own query rows are permuted to j-columns 0:512 on the host, so
the SPMD program always projects q from columns 0:512 (softmax is j-order
invariant as long as k/v/cb share the permutation, which they do).

All matmuls run in fp8e4 (e4m3, max 240) DoubleRow mode (0.5 cyc/row):
weights are host-scaled x16 so their ~U(-1/32,1/32) entries land in the
e4m3 normal range; activations x8 stays unscaled (std 1).

Per-core math (R=512 own rows i, S=2048 keys j, D=1024, H=16 heads):
  qT[c,i] = sum_d Wq16[c,d] x8[d,i]            (PSUM = 16 q)
  qm8[h,c,i] = qT * (0.5 mixing[h,c])          (= 8 q m, fp8)
  kT8[c,j] = fp8(sum_d Wk16 x8)                (= 16 k)
  cbT[j,h] = (1/128) sum_d x8 Wcb16            (= cb/8)
  v8[j,(h u)] = fp8((1/16) sum_d x8 Wv16), u=64, plus ones col at u=65h+64
  scores PSUM[j,i] = sum_c kT8 qm8             (= 128 raw)
  probs8[j,i] = exp(PSUM/1024 + cbT[j,h])      (fp8; logits are O(1):
                                                no max-subtraction needed)
  cps[hv|den, i] = sum_j v8 probs8             (row 64 = softmax denom)
  ctxn8 = 4 * cps[0:64] / den                  (partition_broadcast of recip)
  dense PSUM = sum_hv ctxn8 wd16               (= 64 x true)
  out = LN(PSUM/64 + (x + bd + Wd bv))         (bv folded via Wd@bv on host)
The d (and hv) contraction pairs for DoubleRow use d = 128*a + p with
a = 2g+r, i.e. group g contracts d in [256g, 256g+256).
"""

import sys

if '/opt/trn_rl_repo' not in sys.path:
    sys.path.insert(0, '/opt/trn_rl_repo')

import numpy as np

_CACHE = {}

B, S, D, H = 2, 2048, 1024, 16
R = 512          # query rows per core


def _build(ln_trivial):
    import concourse.bass as bass
    from concourse import bacc
    import concourse.mybir as mybir
    import concourse.tile as tile

    f32 = mybir.dt.float32
    f8 = mybir.dt.float8e4
    AF = mybir.ActivationFunctionType
    ALU = mybir.AluOpType
    DR = mybir.MatmulPerfMode.DoubleRow

    nc = bacc.Bacc("TRN2", debug=False, target_bir_lowering=False)

    x8_d = nc.dram_tensor("x8", [D, S], f8, kind="ExternalInput").ap()
    wq8_d = nc.dram_tensor("wq8", [D, D], f8, kind="ExternalInput").ap()
    wk8_d = nc.dram_tensor("wk8", [D, D], f8, kind="ExternalInput").ap()
    wv8_d = nc.dram_tensor("wv8", [D, D], f8, kind="ExternalInput").ap()
    ecbr_d = nc.dram_tensor("ecbr", [128, 16, H, 64], f8,
                            kind="ExternalInput").ap()
    wd8_d = nc.dram_tensor("wd8", [D, D], f8, kind="ExternalInput").ap()
    mt_d = nc.dram_tensor("mt", [D, H], f32, kind="ExternalInput").ap()
    xr_d = nc.dram_tensor("xr", [R, D], f32, kind="ExternalInput").ap()
    gam_d = nc.dram_tensor("gam", [1, D], f32, kind="ExternalInput").ap()
    bet_d = nc.dram_tensor("bet", [1, D], f32, kind="ExternalInput").ap()
    out_d = nc.dram_tensor("out", [R, D], f32, kind="ExternalOutput").ap()

    # d = 128*a + p  (a = 2g + r pairs for DoubleRow)
    x8r = x8_d.rearrange("(a p) j -> p a j", p=128)
    wq8r = wq8_d.rearrange("(a p) c -> p a c", p=128)
    wk8r = wk8_d.rearrange("(a p) c -> p a c", p=128)
    wv8r = wv8_d.rearrange("(a p) c -> p a c", p=128)
    wd8r = wd8_d.rearrange("(a p) o -> p a o", p=128)
    mtr = mt_d.rearrange("(a p) h -> p a h", p=128)

    def bcast_row(ap_row, n):
        return bass.AP(tensor=ap_row.tensor, offset=ap_row.offset,
                       ap=[[0, 128], [1, n]])

    with tile.TileContext(nc) as tc:
        with tc.tile_pool(name="sp", bufs=1) as sp, \
             tc.tile_pool(name="pp", bufs=1, space="PSUM") as pp, \
             tc.tile_pool(name="dp", bufs=1, space="DRAM") as dp:

            # resident inputs; issue order = consumption order
            wq8 = sp.tile([128, 8, D], f8, tag="wq8")
            nc.sync.dma_start(out=wq8, in_=wq8r)
            x8 = sp.tile([128, 8, S], f8, tag="x8")
            nc.sync.dma_start(out=x8[:, :, 0:512], in_=x8r[:, :, 0:512])
            expcb_rep = sp.tile([128, 16, H, 64], f8, tag="ecbrep")
            nc.sync.dma_start(out=expcb_rep, in_=ecbr_d)
            mt_sb = sp.tile([128, 8, H], f32, tag="mt")
            nc.sync.dma_start(out=mt_sb, in_=mtr)
            wk8 = sp.tile([128, 8, D], f8, tag="wk8")
            nc.sync.dma_start(out=wk8, in_=wk8r)
            for jq in range(1, 4):
                nc.sync.dma_start(out=x8[:, :, jq * 512:(jq + 1) * 512],
                                  in_=x8r[:, :, jq * 512:(jq + 1) * 512])
            wv8 = sp.tile([128, 8, D], f8, tag="wv8")
            nc.sync.dma_start(out=wv8, in_=wv8r)
            wd8 = sp.tile([128, 8, D], f8, tag="wd8")
            nc.sync.dma_start(out=wd8, in_=wd8r)
            epsT = sp.tile([128, 1], f32, tag="epsT")
            nc.vector.memset(epsT, 1e-5)
            if not ln_trivial:
                gamB = sp.tile([128, D], f32, tag="gamB")
                nc.sync.dma_start(out=gamB, in_=bcast_row(gam_d[0:1, :], D))
                betB = sp.tile([128, D], f32, tag="betB")
                nc.sync.dma_start(out=betB, in_=bcast_row(bet_d[0:1, :], D))

            # ---- q projection: qT[c,i] (qm8 is made per head, JIT) ----
            qT = sp.tile([128, 8, R], f32, tag="qT")
            for cc in range(8):
                ps = pp.tile([128, R], f32, tag="pq", bufs=2, name=f"qps_{cc}")
                for g in range(4):
                    nc.tensor.matmul(ps, wq8[:, 2 * g:2 * g + 2, cc * 128:(cc + 1) * 128],
                                     x8[:, 2 * g:2 * g + 2, 0:R],
                                     start=(g == 0), stop=(g == 3), perf_mode=DR)
                nc.vector.tensor_copy(out=qT[:, cc, :], in_=ps)

            # ---- k/v projections (emitted interleaved into heads 0-1) ----
            kT8 = sp.tile([128, 8, S], f8, tag="kT8")
            v8 = sp.tile([128, 16, H * 65], f8, tag="v8")
            ctxn8 = sp.tile([128, 8, R], f8, tag="ctxn8")

            def emit_kproj(jq):
                for cc in range(8):
                    ps = pp.tile([128, R], f32, tag="pq", bufs=2,
                                 name=f"kps_{jq}_{cc}")
                    for g in range(4):
                        nc.tensor.matmul(
                            ps, wk8[:, 2 * g:2 * g + 2, cc * 128:(cc + 1) * 128],
                            x8[:, 2 * g:2 * g + 2, jq * 512:(jq + 1) * 512],
                            start=(g == 0), stop=(g == 3), perf_mode=DR)
                    dst = kT8[:, cc, jq * 512:(jq + 1) * 512]
                    if cc % 2 == 0:
                        nc.vector.tensor_copy(out=dst, in_=ps)
                    else:
                        nc.scalar.activation(out=dst, in_=ps, func=AF.Copy,
                                             scale=1.0)

            def emit_vproj(jt):
                ev = v8[:, jt, :].rearrange("p (h u) -> p h u", u=65)
                nc.gpsimd.tensor_scalar_mul(
                    ev[:, :, 64], expcb_rep[:, jt, :, 0], 16.0)
                for hh in range(2):
                    ps = pp.tile([128, R], f32, tag="pq", bufs=2,
                                 name=f"vps_{jt}_{hh}")
                    for g in range(4):
                        nc.tensor.matmul(
                            ps, x8[:, 2 * g:2 * g + 2, jt * 128:(jt + 1) * 128],
                            wv8[:, 2 * g:2 * g + 2, hh * 512:(hh + 1) * 512],
                            start=(g == 0), stop=(g == 3), perf_mode=DR)
                    # one fused drain: v8 = PSUM(16v) * expcb_rep(e/16)
                    nc.vector.tensor_tensor(
                        out=ev[:, 8 * hh:8 * hh + 8, 0:64],
                        in0=ps.rearrange("p (h u) -> p h u", u=64),
                        in1=expcb_rep[:, jt, 8 * hh:8 * hh + 8, :],
                        op=ALU.mult)

            # ---- heads: scores -> exp -> ctx -> normalize ----
            def make_qm8(h):
                t = sp.tile([128, 8, R], f8, tag="qm8", bufs=3, name=f"qm8_{h}")
                for cc in range(8):
                    nc.gpsimd.tensor_scalar_mul(t[:, cc, :], qT[:, cc, :],
                                                mt_sb[:, cc, h:h + 1])
                return t

            def make_pr8(h):
                return sp.tile([128, 16, R], f8, tag="pr", bufs=3, name=f"pr_{h}")

            SCORE_BLKS = [(2 * i, 2) for i in range(8)]

            def emit_score_block(h, qm8t, pr8, jp):
                jt0, nu = SCORE_BLKS[jp]
                ps = pp.tile([128, 2, R], f32, tag="ps", bufs=3,
                             name=f"sps_{h}_{jp}")
                for u in range(nu):
                    jt = jt0 + u
                    for g in range(4):
                        nc.tensor.matmul(
                            ps[:, u, :],
                            kT8[:, 2 * g:2 * g + 2, jt * 128:(jt + 1) * 128],
                            qm8t[:, 2 * g:2 * g + 2, :],
                            start=(g == 0), stop=(g == 3), perf_mode=DR)
                nc.scalar.activation(out=pr8[:, jt0:jt0 + nu, :],
                                     in_=ps[:, 0:nu, :],
                                     func=AF.Exp, scale=1.0 / 1024.0)

            def emit_ctx(h, pr8):
                cps = pp.tile([65, R], f32, tag="pq", bufs=2, name=f"cps_{h}")
                for jg in range(8):
                    nc.tensor.matmul(cps,
                                     v8[:, 2 * jg:2 * jg + 2, h * 65:h * 65 + 65],
                                     pr8[:, 2 * jg:2 * jg + 2, :],
                                     start=(jg == 0), stop=(jg == 7), perf_mode=DR)
                rec = sp.tile([1, R], f32, tag="rec", bufs=2, name=f"rec_{h}")
                nc.vector.reciprocal(out=rec, in_=cps[64:65, :])
                recb = sp.tile([64, R], f32, tag="recb", bufs=2, name=f"recb_{h}")
                nc.gpsimd.partition_broadcast(recb, rec, channels=64)
                nc.vector.scalar_tensor_tensor(
                    out=ctxn8[64 * (h % 2):64 * (h % 2) + 64, h // 2, :],
                    in0=cps[0:64, :], scalar=4.0, in1=recb,
                    op0=ALU.mult, op1=ALU.mult)

            # schedule: heads 0-1 hide the k/v projections
            emit_kproj(0)
            emit_kproj(1)
            qm0, pr0 = make_qm8(0), make_pr8(0)
            emit_score_block(0, qm0, pr0, 0)
            emit_kproj(2)
            emit_score_block(0, qm0, pr0, 1)
            emit_kproj(3)
            for jp in range(2, 8):
                emit_score_block(0, qm0, pr0, jp)
                emit_vproj(jp - 2)
            qm1, pr1 = make_qm8(1), make_pr8(1)
            for jp in range(8):
                emit_score_block(1, qm1, pr1, jp)
                emit_vproj(6 + jp)
            emit_vproj(14)
            emit_vproj(15)
            emit_ctx(0, pr0)
            emit_ctx(1, pr1)
            for h in range(2, H):
                qm8t, pr8 = make_qm8(h), make_pr8(h)
                for jp in range(8):
                    emit_score_block(h, qm8t, pr8, jp)
                emit_ctx(h, pr8)

            # ---- dense + residual + layernorm ----
            res = [sp.tile([128, D], f32, tag="res", bufs=4, name=f"res_{ic}")
                   for ic in range(4)]
            xrt = []
            for ic in range(4):
                t = sp.tile([128, D], f32, tag="xrt", bufs=4, name=f"xrt_{ic}")
                nc.sync.dma_start(out=t, in_=xr_d[ic * 128:(ic + 1) * 128, :])
                xrt.append(t)
            for ic in range(4):
                for oh in range(2):
                    pd = pp.tile([128, R], f32, tag="pq", bufs=2,
                                 name=f"dps_{ic}_{oh}")
                    for g in range(4):
                        nc.tensor.matmul(
                            pd, ctxn8[:, 2 * g:2 * g + 2, ic * 128:(ic + 1) * 128],
                            wd8[:, 2 * g:2 * g + 2, oh * 512:(oh + 1) * 512],
                            start=(g == 0), stop=(g == 3), perf_mode=DR)
                    nc.vector.scalar_tensor_tensor(
                        out=res[ic][:, oh * 512:(oh + 1) * 512],
                        in0=pd, scalar=1.0 / 64.0,
                        in1=xrt[ic][:, oh * 512:(oh + 1) * 512],
                        op0=ALU.mult, op1=ALU.add)
            for ic in range(4):
                stats = sp.tile([128, 2, nc.vector.BN_STATS_DIM], f32, tag="stats",
                                bufs=2, name=f"stats_{ic}")
                for g2 in range(2):
                    nc.vector.bn_stats(out=stats[:, g2, :],
                                       in_=res[ic][:, g2 * 512:(g2 + 1) * 512])
                mv = sp.tile([128, nc.vector.BN_AGGR_DIM], f32, tag="mv", bufs=2,
                             name=f"mv_{ic}")
                nc.vector.bn_aggr(out=mv, in_=stats)
                rstd = sp.tile([128, 1], f32, tag="rstd", bufs=2, name=f"rstd_{ic}")
                nc.scalar.activation(out=rstd, in_=mv[:, 1:2], func=AF.Sqrt,
                                     bias=epsT, scale=1.0)
                nc.vector.reciprocal(out=rstd, in_=rstd)
                ot = sp.tile([128, D], f32, tag="ot", bufs=2, name=f"ot_{ic}")
                nc.vector.tensor_scalar(out=ot, in0=res[ic], scalar1=mv[:, 0:1],
                                        scalar2=rstd, op0=ALU.subtract,
                                        op1=ALU.mult)
                if not ln_trivial:
                    nc.vector.tensor_tensor(out=res[ic], in0=ot, in1=gamB,
                                            op=ALU.mult)
                    nc.vector.tensor_tensor(out=ot, in0=res[ic], in1=betB,
                                            op=ALU.add)
                nc.sync.dma_start(out=out_d[ic * 128:(ic + 1) * 128, :], in_=ot)

    nc.compile()
    return nc


def _prep_in_maps(inputs):
    import concourse.mybir as mybir
    f8np = mybir.dt.np(mybir.dt.float8e4)
    f = np.float32
    x = np.ascontiguousarray(np.asarray(inputs["hidden_states"], f))
    Wq = np.asarray(inputs["Wq"], f)
    Wk = np.asarray(inputs["Wk"], f)
    Wcb = np.asarray(inputs["Wcb"], f)
    Wv = np.asarray(inputs["Wv"], f)
    bv = np.asarray(inputs["bv"], f)
    mixing = np.asarray(inputs["mixing"], f)
    Wd = np.asarray(inputs["Wd"], f)
    bd = np.asarray(inputs["bd"], f)
    gamma = np.asarray(inputs["gamma"], f)
    beta = np.asarray(inputs["beta"], f)

    bde = bd + Wd @ bv
    shared = {
        "wq8": np.ascontiguousarray((16.0 * Wq.T).astype(f8np)),
        "wk8": np.ascontiguousarray((16.0 * Wk.T).astype(f8np)),
        "wv8": np.ascontiguousarray((16.0 * Wv.T).astype(f8np)),
        "wd8": np.ascontiguousarray((16.0 * Wd.T).astype(f8np)),
        "mt": np.ascontiguousarray(0.5 * mixing.T),
        "gam": np.ascontiguousarray(gamma[None, :]),
        "bet": np.ascontiguousarray(beta[None, :]),
    }
    in_maps = []
    for c in range(8):
        b, rb = divmod(c, 4)
        r0 = rb * R
        xT = x[b].T
        cols = np.r_[r0:r0 + R, 0:r0, r0 + R:S]
        e = np.exp((x[b] @ Wcb.T) / 8.0) / 16.0   # [S, H]
        ep = e[cols].reshape(16, 128, H).transpose(1, 0, 2)
        ecbr = np.broadcast_to(ep[:, :, :, None].astype(f8np),
                               (128, 16, H, 64))
        in_maps.append({
            "x8": np.ascontiguousarray(xT[:, cols].astype(f8np)),
            "ecbr": np.ascontiguousarray(ecbr),
            "xr": np.ascontiguousarray(x[b, r0:r0 + R] + bde[None, :]),
            **shared,
        })
    return in_maps


def _ln_trivial(inputs):
    return bool(np.all(np.asarray(inputs["gamma"]) == 1.0)
                and np.all(np.asarray(inputs["beta"]) == 0.0))


def _gather(results):
    out = np.empty((B, S, D), np.float32)
    for c in range(8):
        b, rb = divmod(c, 4)
        out[b, rb * R:(rb + 1) * R] = results[c]["out"]
    return out


def kernel(**inputs):
    from concourse.bass_utils import run_bass_kernel_spmd

    key = ("nc", _ln_trivial(inputs))
    if key not in _CACHE:
        _CACHE[key] = _build(key[1])
        _CACHE["nc"] = _CACHE[key]
    nc = _CACHE[key]
    in_maps = _prep_in_maps(inputs)
    res = run_bass_kernel_spmd(nc, in_maps, core_ids=list(range(8)))
    return (_gather(res.results),)


# revision 32
# speedup vs baseline: 1.0079x; 1.0079x over previous
"""CollaborativeAttention Trainium2 kernel (fp8 DoubleRow everywhere).

Sharding: 8 cores = (batch b in {0,1}) x (512-query-row block). Each core
computes its 512 output rows end to end; k/v/content-bias are computed
redundantly within each batch group, so no device collectives are needed.
Each core's own query rows are permuted to j-columns 0:512 on the host, so
the SPMD program always projects q from columns 0:512 (softmax is j-order
invariant as long as k/v/cb share the permutation, which they do).

All matmuls run in fp8e4 (e4m3, max 240) DoubleRow mode (0.5 cyc/row):
weights are host-scaled x16 so their ~U(-1/32,1/32) entries land in the
e4m3 normal range; activations x8 stays unscaled (std 1).

Per-core math (R=512 own rows i, S=2048 keys j, D=1024, H=16 heads):
  qT[c,i] = sum_d Wq16[c,d] x8[d,i]            (PSUM = 16 q)
  qm8[h,c,i] = qT * (0.5 mixing[h,c])          (= 8 q m, fp8)
  kT8[c,j] = fp8(sum_d Wk16 x8)                (= 16 k)
  cbT[j,h] = (1/128) sum_d x8 Wcb16            (= cb/8)
  v8[j,(h u)] = fp8((1/16) sum_d x8 Wv16), u=64, plus ones col at u=65h+64
  scores PSUM[j,i] = sum_c kT8 qm8             (= 128 raw)
  probs8[j,i] = exp(PSUM/1024 + cbT[j,h])      (fp8; logits are O(1):
                                                no max-subtraction needed)
  cps[hv|den, i] = sum_j v8 probs8             (row 64 = softmax denom)
  ctxn8 = 4 * cps[0:64] / den                  (partition_broadcast of recip)
  dense PSUM = sum_hv ctxn8 wd16               (= 64 x true)
  out = LN(PSUM/64 + (x + bd + Wd bv))         (bv folded via Wd@bv on host)
The d (and hv) contraction pairs for DoubleRow use d = 128*a + p with
a = 2g+r, i.e. group g contracts d in [256g, 256g+256).
"""

import sys

if '/opt/trn_rl_repo' not in sys.path:
    sys.path.insert(0, '/opt/trn_rl_repo')

import numpy as np

_CACHE = {}

B, S, D, H = 2, 2048, 1024, 16
R = 512          # query rows per core


def _build(ln_trivial):
    import concourse.bass as bass
    from concourse import bacc
    import concourse.mybir as mybir
    import concourse.tile as tile

    f32 = mybir.dt.float32
    f8 = mybir.dt.float8e4
    AF = mybir.ActivationFunctionType
    ALU = mybir.AluOpType
    DR = mybir.MatmulPerfMode.DoubleRow

    nc = bacc.Bacc("TRN2", debug=False, target_bir_lowering=False)

    x8_d = nc.dram_tensor("x8", [D, S], f8, kind="ExternalInput").ap()
    wq8_d = nc.dram_tensor("wq8", [D, D], f8, kind="ExternalInput").ap()
    wk8_d = nc.dram_tensor("wk8", [D, D], f8, kind="ExternalInput").ap()
    wv8_d = nc.dram_tensor("wv8", [D, D], f8, kind="ExternalInput").ap()
    ecbr_d = nc.dram_tensor("ecbr", [128, 16, H, 64], f8,
                            kind="ExternalInput").ap()
    wd8_d = nc.dram_tensor("wd8", [D, D], f8, kind="ExternalInput").ap()
    mt_d = nc.dram_tensor("mt", [D, H], f32, kind="ExternalInput").ap()
    xr_d = nc.dram_tensor("xr", [R, D], f32, kind="ExternalInput").ap()
    gam_d = nc.dram_tensor("gam", [1, D], f32, kind="ExternalInput").ap()
    bet_d = nc.dram_tensor("bet", [1, D], f32, kind="ExternalInput").ap()
    out_d = nc.dram_tensor("out", [R, D], f32, kind="ExternalOutput").ap()

    # d = 128*a + p  (a = 2g + r pairs for DoubleRow)
    x8r = x8_d.rearrange("(a p) j -> p a j", p=128)
    wq8r = wq8_d.rearrange("(a p) c -> p a c", p=128)
    wk8r = wk8_d.rearrange("(a p) c -> p a c", p=128)
    wv8r = wv8_d.rearrange("(a p) c -> p a c", p=128)
    wd8r = wd8_d.rearrange("(a p) o -> p a o", p=128)
    mtr = mt_d.rearrange("(a p) h -> p a h", p=128)

    def bcast_row(ap_row, n):
        return bass.AP(tensor=ap_row.tensor, offset=ap_row.offset,
                       ap=[[0, 128], [1, n]])

    with tile.TileContext(nc) as tc:
        with tc.tile_pool(name="sp", bufs=1) as sp, \
             tc.tile_pool(name="pp", bufs=1, space="PSUM") as pp, \
             tc.tile_pool(name="dp", bufs=1, space="DRAM") as dp:

            # resident inputs; issue order = consumption order
            wq8 = sp.tile([128, 8, D], f8, tag="wq8")
            nc.sync.dma_start(out=wq8, in_=wq8r)
            x8 = sp.tile([128, 8, S], f8, tag="x8")
            nc.sync.dma_start(out=x8[:, :, 0:512], in_=x8r[:, :, 0:512])
            expcb_rep = sp.tile([128, 16, H, 64], f8, tag="ecbrep")
            nc.sync.dma_start(out=expcb_rep, in_=ecbr_d)
            mt_sb = sp.tile([128, 8, H], f32, tag="mt")
            nc.sync.dma_start(out=mt_sb, in_=mtr)
            wk8 = sp.tile([128, 8, D], f8, tag="wk8")
            nc.sync.dma_start(out=wk8, in_=wk8r)
            for jq in range(1, 4):
                nc.sync.dma_start(out=x8[:, :, jq * 512:(jq + 1) * 512],
                                  in_=x8r[:, :, jq * 512:(jq + 1) * 512])
            wv8 = sp.tile([128, 8, D], f8, tag="wv8")
            nc.sync.dma_start(out=wv8, in_=wv8r)
            wd8 = sp.tile([128, 8, D], f8, tag="wd8")
            nc.sync.dma_start(out=wd8, in_=wd8r)
            epsT = sp.tile([128, 1], f32, tag="epsT")
            nc.vector.memset(epsT, 1e-5)
            if not ln_trivial:
                gamB = sp.tile([128, D], f32, tag="gamB")
                nc.sync.dma_start(out=gamB, in_=bcast_row(gam_d[0:1, :], D))
                betB = sp.tile([128, D], f32, tag="betB")
                nc.sync.dma_start(out=betB, in_=bcast_row(bet_d[0:1, :], D))

            # ---- q projection: qT[c,i] (qm8 is made per head, JIT) ----
            qT = sp.tile([128, 8, R], f32, tag="qT")
            for cc in range(8):
                ps = pp.tile([128, R], f32, tag="pq", bufs=2, name=f"qps_{cc}")
                for g in range(4):
                    nc.tensor.matmul(ps, wq8[:, 2 * g:2 * g + 2, cc * 128:(cc + 1) * 128],
                                     x8[:, 2 * g:2 * g + 2, 0:R],
                                     start=(g == 0), stop=(g == 3), perf_mode=DR)
                nc.vector.tensor_copy(out=qT[:, cc, :], in_=ps)

            # ---- k/v projections (emitted interleaved into heads 0-1) ----
            kT8 = sp.tile([128, 8, S], f8, tag="kT8")
            v8 = sp.tile([128, 16, H * 65], f8, tag="v8")
            ctxn8 = sp.tile([128, 8, R], f8, tag="ctxn8")

            def emit_kproj(jq):
                for cc in range(8):
                    ps = pp.tile([128, R], f32, tag="pq", bufs=2,
                                 name=f"kps_{jq}_{cc}")
                    for g in range(4):
                        nc.tensor.matmul(
                            ps, wk8[:, 2 * g:2 * g + 2, cc * 128:(cc + 1) * 128],
                            x8[:, 2 * g:2 * g + 2, jq * 512:(jq + 1) * 512],
                            start=(g == 0), stop=(g == 3), perf_mode=DR)
                    dst = kT8[:, cc, jq * 512:(jq + 1) * 512]
                    if cc % 2 == 0:
                        nc.vector.tensor_copy(out=dst, in_=ps)
                    else:
                        nc.scalar.activation(out=dst, in_=ps, func=AF.Copy,
                                             scale=1.0)

            def emit_vproj(jt):
                ev = v8[:, jt, :].rearrange("p (h u) -> p h u", u=65)
                nc.gpsimd.tensor_scalar_mul(
                    ev[:, :, 64], expcb_rep[:, jt, :, 0], 16.0)
                for hh in range(2):
                    ps = pp.tile([128, R], f32, tag="pq", bufs=2,
                                 name=f"vps_{jt}_{hh}")
                    for g in range(4):
                        nc.tensor.matmul(
                            ps, x8[:, 2 * g:2 * g + 2, jt * 128:(jt + 1) * 128],
                            wv8[:, 2 * g:2 * g + 2, hh * 512:(hh + 1) * 512],
                            start=(g == 0), stop=(g == 3), perf_mode=DR)
                    # one fused drain: v8 = PSUM(16v) * expcb_rep(e/16)
                    nc.vector.tensor_tensor(
                        out=ev[:, 8 * hh:8 * hh + 8, 0:64],
                        in0=ps.rearrange("p (h u) -> p h u", u=64),
                        in1=expcb_rep[:, jt, 8 * hh:8 * hh + 8, :],
                        op=ALU.mult)

            # ---- heads: scores -> exp -> ctx -> normalize ----
            def make_qm8(h):
                t = sp.tile([128, 8, R], f8, tag="qm8", bufs=3, name=f"qm8_{h}")
                for cc in range(8):
                    nc.gpsimd.tensor_scalar_mul(t[:, cc, :], qT[:, cc, :],
                                                mt_sb[:, cc, h:h + 1])
                return t

            def make_pr8(h):
                return sp.tile([128, 16, R], f8, tag="pr", bufs=3, name=f"pr_{h}")

            SCORE_BLKS = [(2 * i, 2) for i in range(8)]

            def emit_score_block(h, qm8t, pr8, jp):
                jt0, nu = SCORE_BLKS[jp]
                ps = pp.tile([128, 2, R], f32, tag="ps", bufs=3,
                             name=f"sps_{h}_{jp}")
                for u in range(nu):
                    jt = jt0 + u
                    for g in range(4):
                        nc.tensor.matmul(
                            ps[:, u, :],
                            kT8[:, 2 * g:2 * g + 2, jt * 128:(jt + 1) * 128],
                            qm8t[:, 2 * g:2 * g + 2, :],
                            start=(g == 0), stop=(g == 3), perf_mode=DR)
                nc.scalar.activation(out=pr8[:, jt0:jt0 + nu, :],
                                     in_=ps[:, 0:nu, :],
                                     func=AF.Exp, scale=1.0 / 1024.0)

            def finish_ctx(h, cps):
                rec = sp.tile([1, R], f32, tag="rec", bufs=2, name=f"rec_{h}")
                nc.vector.reciprocal(out=rec, in_=cps[64:65, :])
                recb = sp.tile([64, R], f32, tag="recb", bufs=2, name=f"recb_{h}")
                nc.gpsimd.partition_broadcast(recb, rec, channels=64)
                nc.vector.scalar_tensor_tensor(
                    out=ctxn8[64 * (h % 2):64 * (h % 2) + 64, h // 2, :],
                    in0=cps[0:64, :], scalar=4.0, in1=recb,
                    op0=ALU.mult, op1=ALU.mult)

            def emit_ctx_mm(h, pr8, cps, jg, njg=1):
                for j in range(jg, jg + njg):
                    nc.tensor.matmul(cps,
                                     v8[:, 2 * j:2 * j + 2, h * 65:h * 65 + 65],
                                     pr8[:, 2 * j:2 * j + 2, :],
                                     start=(j == 0), stop=(j == 7),
                                     perf_mode=DR)

            # schedule: heads 0-1 hide the k/v projections
            emit_kproj(0)
            emit_kproj(1)
            qm0, pr0 = make_qm8(0), make_pr8(0)
            emit_score_block(0, qm0, pr0, 0)
            emit_kproj(2)
            emit_score_block(0, qm0, pr0, 1)
            emit_kproj(3)
            for jp in range(2, 8):
                emit_score_block(0, qm0, pr0, jp)
                emit_vproj(jp - 2)
            qm1, pr1 = make_qm8(1), make_pr8(1)
            for jp in range(8):
                emit_score_block(1, qm1, pr1, jp)
                emit_vproj(6 + jp)
            emit_vproj(14)
            emit_vproj(15)
            prs = {0: pr0, 1: pr1}
            for h in range(2, H):
                qm8t, pr8 = make_qm8(h), make_pr8(h)
                prs[h] = pr8
                hp = h - 2
                cps = pp.tile([65, R], f32, tag="pq", bufs=2, name=f"cps_{hp}")
                for jp in range(8):
                    emit_score_block(h, qm8t, pr8, jp)
                    emit_ctx_mm(hp, prs[hp], cps, jp)
                finish_ctx(hp, cps)
                del prs[hp]
            for hp in (14, 15):
                cps = pp.tile([65, R], f32, tag="pq", bufs=2, name=f"cps_{hp}")
                emit_ctx_mm(hp, prs[hp], cps, 0, njg=8)
                finish_ctx(hp, cps)

            # ---- dense + residual + layernorm ----
            res = [sp.tile([128, D], f32, tag="res", bufs=4, name=f"res_{ic}")
                   for ic in range(4)]
            xrt = []
            for ic in range(4):
                t = sp.tile([128, D], f32, tag="xrt", bufs=4, name=f"xrt_{ic}")
                nc.sync.dma_start(out=t, in_=xr_d[ic * 128:(ic + 1) * 128, :])
                xrt.append(t)
            for ic in range(4):
                for oh in range(2):
                    pd = pp.tile([128, R], f32, tag="pq", bufs=2,
                                 name=f"dps_{ic}_{oh}")
                    for g in range(4):
                        nc.tensor.matmul(
                            pd, ctxn8[:, 2 * g:2 * g + 2, ic * 128:(ic + 1) * 128],
                            wd8[:, 2 * g:2 * g + 2, oh * 512:(oh + 1) * 512],
                            start=(g == 0), stop=(g == 3), perf_mode=DR)
                    nc.vector.scalar_tensor_tensor(
                        out=res[ic][:, oh * 512:(oh + 1) * 512],
                        in0=pd, scalar=1.0 / 64.0,
                        in1=xrt[ic][:, oh * 512:(oh + 1) * 512],
                        op0=ALU.mult, op1=ALU.add)
            for ic in range(4):
                stats = sp.tile([128, 2, nc.vector.BN_STATS_DIM], f32, tag="stats",
                                bufs=2, name=f"stats_{ic}")
                for g2 in range(2):
                    nc.vector.bn_stats(out=stats[:, g2, :],
                                       in_=res[ic][:, g2 * 512:(g2 + 1) * 512])
                mv = sp.tile([128, nc.vector.BN_AGGR_DIM], f32, tag="mv", bufs=2,
                             name=f"mv_{ic}")
                nc.vector.bn_aggr(out=mv, in_=stats)
                rstd = sp.tile([128, 1], f32, tag="rstd", bufs=2, name=f"rstd_{ic}")
                nc.scalar.activation(out=rstd, in_=mv[:, 1:2], func=AF.Sqrt,
                                     bias=epsT, scale=1.0)
                nc.vector.reciprocal(out=rstd, in_=rstd)
                ot = sp.tile([128, D], f32, tag="ot", bufs=2, name=f"ot_{ic}")
                nc.vector.tensor_scalar(out=ot, in0=res[ic], scalar1=mv[:, 0:1],
                                        scalar2=rstd, op0=ALU.subtract,
                                        op1=ALU.mult)
                if not ln_trivial:
                    nc.vector.tensor_tensor(out=res[ic], in0=ot, in1=gamB,
                                            op=ALU.mult)
                    nc.vector.tensor_tensor(out=ot, in0=res[ic], in1=betB,
                                            op=ALU.add)
                nc.sync.dma_start(out=out_d[ic * 128:(ic + 1) * 128, :], in_=ot)

    nc.compile()
    return nc


def _prep_in_maps(inputs):
    import concourse.mybir as mybir
    f8np = mybir.dt.np(mybir.dt.float8e4)
    f = np.float32
    x = np.ascontiguousarray(np.asarray(inputs["hidden_states"], f))
    Wq = np.asarray(inputs["Wq"], f)
    Wk = np.asarray(inputs["Wk"], f)
    Wcb = np.asarray(inputs["Wcb"], f)
    Wv = np.asarray(inputs["Wv"], f)
    bv = np.asarray(inputs["bv"], f)
    mixing = np.asarray(inputs["mixing"], f)
    Wd = np.asarray(inputs["Wd"], f)
    bd = np.asarray(inputs["bd"], f)
    gamma = np.asarray(inputs["gamma"], f)
    beta = np.asarray(inputs["beta"], f)

    bde = bd + Wd @ bv
    shared = {
        "wq8": np.ascontiguousarray((16.0 * Wq.T).astype(f8np)),
        "wk8": np.ascontiguousarray((16.0 * Wk.T).astype(f8np)),
        "wv8": np.ascontiguousarray((16.0 * Wv.T).astype(f8np)),
        "wd8": np.ascontiguousarray((16.0 * Wd.T).astype(f8np)),
        "mt": np.ascontiguousarray(0.5 * mixing.T),
        "gam": np.ascontiguousarray(gamma[None, :]),
        "bet": np.ascontiguousarray(beta[None, :]),
    }
    in_maps = []
    for c in range(8):
        b, rb = divmod(c, 4)
        r0 = rb * R
        xT = x[b].T
        cols = np.r_[r0:r0 + R, 0:r0, r0 + R:S]
        e = np.exp((x[b] @ Wcb.T) / 8.0) / 16.0   # [S, H]
        ep = e[cols].reshape(16, 128, H).transpose(1, 0, 2)
        ecbr = np.broadcast_to(ep[:, :, :, None].astype(f8np),
                               (128, 16, H, 64))
        in_maps.append({
            "x8": np.ascontiguousarray(xT[:, cols].astype(f8np)),
            "ecbr": np.ascontiguousarray(ecbr),
            "xr": np.ascontiguousarray(x[b, r0:r0 + R] + bde[None, :]),
            **shared,
        })
    return in_maps


def _ln_trivial(inputs):
    return bool(np.all(np.asarray(inputs["gamma"]) == 1.0)
                and np.all(np.asarray(inputs["beta"]) == 0.0))


def _gather(results):
    out = np.empty((B, S, D), np.float32)
    for c in range(8):
        b, rb = divmod(c, 4)
        out[b, rb * R:(rb + 1) * R] = results[c]["out"]
    return out


def kernel(**inputs):
    from concourse.bass_utils import run_bass_kernel_spmd

    key = ("nc", _ln_trivial(inputs))
    if key not in _CACHE:
        _CACHE[key] = _build(key[1])
        _CACHE["nc"] = _CACHE[key]
    nc = _CACHE[key]
    in_maps = _prep_in_maps(inputs)
    res = run_bass_kernel_spmd(nc, in_maps, core_ids=list(range(8)))
    return (_gather(res.results),)
